# revision 10
# baseline (speedup 1.0000x reference)
"""Gated attention layer (B=8, S=2048, D=1024) on 8 Trainium2 NeuronCores.

Sharding: data-parallel over batch B — core b computes batch element b
end-to-end (weights replicated). No collectives.

Per-core dataflow:
  phase 1: transpose inputs on PE; project V (natural layout, bf16),
           K^T (SBUF-resident fp32), Q^T and gate=sigmoid(Xq@Wg+bg)
           (DRAM scratch).  Matmuls run as fp32r (full PE rate at N>=256).
  phase 2: per 128-row q tile: scores = Q^T slice x K^T (PSUM, fp32r),
           softmax along free axis (DVE max-reduce + ACT exp with fused
           -max bias and accumulated row sum), PE-transpose the bf16
           attention tile, ctx = attnT x V (bf16), eviction fused with
           1/sum normalization and the gate multiply, PE-transpose ctxg,
           out = ctxgT x Wo (fp32r) + DMA to HBM.
"""

import numpy as np

import concourse.bass as bass
import concourse.tile as tile
from concourse import bacc, mybir
from concourse.bass_utils import run_bass_kernel_spmd
from concourse.masks import make_identity

B, S, D = 8, 2048, 1024
P = 128
DK = D // P      # 8 contraction chunks of 128
ST = S // P      # 16 seq tiles of 128
NB = S // 512    # 4 scores banks of 512

F32 = mybir.dt.float32
F32R = mybir.dt.float32r
BF16 = mybir.dt.bfloat16
AX = mybir.AxisListType
ALU = mybir.AluOpType
ACTF = mybir.ActivationFunctionType


def _mm(nc, out, lhsT, rhs, start, stop, r=False):
    nc.tensor.matmul(out, lhsT, rhs, start=start, stop=stop)


def build_program(zero_bias: bool, debug: bool = False):
    nc = bacc.Bacc(None, target_bir_lowering=False, debug=debug)

    xq = nc.dram_tensor("xq", [S, D], F32, kind="ExternalInput")
    xk = nc.dram_tensor("xk", [S, D], F32, kind="ExternalInput")
    xv = nc.dram_tensor("xv", [S, D], F32, kind="ExternalInput")
    ws = {
        n: nc.dram_tensor(n, [D, D], F32, kind="ExternalInput")
        for n in ("wq", "wk", "wv", "wg", "wo")
    }
    bs = None
    if not zero_bias:
        bs = {
            n: nc.dram_tensor(n, [D], F32, kind="ExternalInput")
            for n in ("bq", "bk", "bv", "bg", "bo")
        }
    out = nc.dram_tensor("out", [S, D], F32, kind="ExternalOutput")

    with tile.TileContext(nc) as tc:
        _body(tc, xq, xk, xv, ws, bs, out)
    nc.compile()
    return nc


def _bcast(ap_1d, parts):
    """view a [D] dram AP as [parts, D] with partition step 0 (DMA broadcast)"""
    return bass.AP(tensor=ap_1d.tensor, offset=ap_1d.offset,
                   ap=[[0, parts]] + list(ap_1d.ap))


def _body(tc, xq, xk, xv, ws, bs, out):
    nc = tc.nc
    from contextlib import ExitStack

    with ExitStack() as ctx:
        ep = ctx.enter_context

        dram = ep(tc.tile_pool(name="dram", bufs=1, space="DRAM"))
        qt_dram = dram.tile([P, DK, S], F32)       # Q^T scratch: [d, s]
        gate_dram = dram.tile([ST, P, D], F32)     # gate scratch: [s, d]

        const = ep(tc.tile_pool(name="const", bufs=1))
        ident_f = const.tile([P, P], F32)
        make_identity(nc, ident_f)
        ident_b = const.tile([P, P], BF16)
        make_identity(nc, ident_b)

        # bias tiles: per-partition [P, DK] for d_out-partition layouts,
        # broadcast [P, D] for d_out-free layouts
        bias_pp = {}
        bias_bc = {}
        if bs is not None:
            for n in ("bq", "bk"):
                t = const.tile([P, DK], F32)
                nc.sync.dma_start(
                    out=t, in_=bs[n][:].rearrange("(m p) -> p m", p=P))
                bias_pp[n] = t
            for n in ("bv", "bg"):
                t = const.tile([P, D], F32)
                nc.sync.dma_start(out=t, in_=_bcast(bs[n][:], P))
                bias_bc[n] = t

        # ---- long-lived SBUF residents ----
        kv_pool = ep(tc.tile_pool(name="kv", bufs=1))
        kT_sb = kv_pool.tile([P, DK, S], F32R)      # K^T  (64 KiB/part)
        v_sb = kv_pool.tile([P, ST, D], BF16)      # V natural (32 KiB/part)

        # =================== phase 1 ===================
        with tc.tile_pool(name="wchunk", bufs=8) as w_pool, \
             tc.tile_pool(name="evict", bufs=3, space="PSUM") as evict_pool, \
             tc.tile_pool(name="tp", bufs=2, space="PSUM") as tp_pool:

            def load_w(name):
                tiles = []
                for k in range(DK):
                    wt = w_pool.tile([P, D], F32R, tag="wchunk")
                    nc.sync.dma_start(
                        out=wt, in_=ws[name][k * P:(k + 1) * P, :].bitcast(F32R))
                    tiles.append(wt)
                return tiles

            def transpose_x(x_pool, x_dram, xT_sb, s):
                """load X rows [s*128:(s+1)*128, :], PE-transpose into
                xT_sb[:, :, s*128:(s+1)*128]"""
                xt = x_pool.tile([P, D], F32, tag="xstage")
                nc.sync.dma_start(out=xt, in_=x_dram[s * P:(s + 1) * P, :])
                for j in range(2):
                    pst = tp_pool.tile([P, 512], F32, tag="tp")
                    for i in range(4):
                        k = j * 4 + i
                        nc.tensor.transpose(
                            pst[:, i * P:(i + 1) * P],
                            xt[:, k * P:(k + 1) * P], ident_f)
                    dst = xT_sb[:, j * 4:(j + 1) * 4, s * P:(s + 1) * P]
                    nc.vector.tensor_copy(
                        dst, pst.rearrange("p (a b) -> p a b", a=4))

            # -------- phase 1a: V projection (natural layout, bf16) -----
            with tc.tile_pool(name="xvT", bufs=1) as xvT_pool, \
                 tc.tile_pool(name="xstage_v", bufs=2) as x_pool:
                xvT = xvT_pool.tile([P, DK, S], F32R)
                wv = load_w("wv")
                for s in range(ST):
                    transpose_x(x_pool, xv, xvT, s)
                    for n in range(2):
                        ps = evict_pool.tile([P, 512], F32, tag="proj")
                        for k in range(DK):
                            _mm(nc, ps, xvT[:, k, s * P:(s + 1) * P],
                                wv[k][:, n * 512:(n + 1) * 512],
                                start=(k == 0), stop=(k == DK - 1), r=True)
                        dst = v_sb[:, s, n * 512:(n + 1) * 512]
                        if bs is None:
                            nc.vector.tensor_copy(dst, ps)
                        else:
                            nc.vector.scalar_tensor_tensor(
                                out=dst, in0=ps, scalar=1.0,
                                in1=bias_bc["bv"][:, n * 512:(n + 1) * 512],
                                op0=ALU.mult, op1=ALU.add)

            # -------- phase 1b: K^T projection (fp32 resident) ----------
            with tc.tile_pool(name="xkT", bufs=1) as xkT_pool:
                xkT = xkT_pool.tile([P, DK, S], F32R)
                with tc.tile_pool(name="xstage_k", bufs=2) as x_pool:
                    for s in range(ST):
                        transpose_x(x_pool, xk, xkT, s)
                wk = load_w("wk")
                for m in range(DK):
                    for nb in range(NB):
                        ps = evict_pool.tile([P, 512], F32, tag="proj")
                        for k in range(DK):
                            _mm(nc, ps, wk[k][:, m * P:(m + 1) * P],
                                xkT[:, k, nb * 512:(nb + 1) * 512],
                                start=(k == 0), stop=(k == DK - 1), r=True)
                        dst = kT_sb[:, m, nb * 512:(nb + 1) * 512]
                        if bs is None:
                            nc.vector.tensor_copy(dst, ps)
                        else:
                            nc.vector.tensor_scalar_add(
                                dst, ps, bias_pp["bk"][:, m:m + 1])

            # -------- phase 1c: Q^T and gate -> DRAM scratch ------------
            with tc.tile_pool(name="xqT", bufs=1) as xqT_pool:
                xqT = xqT_pool.tile([P, DK, S], F32R)
                with tc.tile_pool(name="xstage_q", bufs=2) as x_pool:
                    for s in range(ST):
                        transpose_x(x_pool, xq, xqT, s)
                with tc.tile_pool(name="stage1c", bufs=3) as st_pool:
                    wq = load_w("wq")
                    for m in range(DK):
                        for nb in range(NB):
                            ps = evict_pool.tile([P, 512], F32, tag="proj")
                            for k in range(DK):
                                _mm(nc, ps, wq[k][:, m * P:(m + 1) * P],
                                    xqT[:, k, nb * 512:(nb + 1) * 512],
                                    start=(k == 0), stop=(k == DK - 1), r=True)
                            stg = st_pool.tile([P, 512], F32, tag="qstage")
                            if bs is None:
                                nc.vector.tensor_copy(stg, ps)
                            else:
                                nc.vector.tensor_scalar_add(
                                    stg, ps, bias_pp["bq"][:, m:m + 1])
                            nc.sync.dma_start(
                                out=qt_dram[:, m, nb * 512:(nb + 1) * 512], in_=stg)

                    wg = load_w("wg")
                    for s in range(ST):
                        for n in range(2):
                            ps = evict_pool.tile([P, 512], F32, tag="proj")
                            for k in range(DK):
                                _mm(nc, ps, xqT[:, k, s * P:(s + 1) * P],
                                    wg[k][:, n * 512:(n + 1) * 512],
                                    start=(k == 0), stop=(k == DK - 1), r=True)
                            if bs is not None:
                                nc.vector.tensor_tensor(
                                    out=ps, in0=ps,
                                    in1=bias_bc["bg"][:, n * 512:(n + 1) * 512],
                                    op=ALU.add)
                            stg = st_pool.tile([P, 512], F32, tag="qstage")
                            nc.scalar.activation(stg, ps, ACTF.Sigmoid)
                            nc.sync.dma_start(
                                out=gate_dram[s, :, n * 512:(n + 1) * 512], in_=stg)

        # =================== phase 2 ===================
        wo_pool = ep(tc.tile_pool(name="wo", bufs=1))
        wo_sb = wo_pool.tile([P, DK, D], F32R)
        for k in range(DK):
            nc.sync.dma_start(out=wo_sb[:, k, :],
                              in_=ws["wo"][k * P:(k + 1) * P, :].bitcast(F32R))
        if bs is not None:
            t = wo_pool.tile([P, D], F32)
            nc.sync.dma_start(out=t, in_=_bcast(bs["bo"][:], P))
            bias_bc["bo"] = t

        p2 = ep(tc.tile_pool(name="p2", bufs=2))
        stats = ep(tc.tile_pool(name="stats", bufs=2))
        ps_a = ep(tc.tile_pool(name="ps_a", bufs=5, space="PSUM"))
        ps_b = ep(tc.tile_pool(name="ps_b", bufs=3, space="PSUM"))

        for t in range(ST):
            qt_sb = p2.tile([P, DK, P], F32R, tag="qt")
            nc.sync.dma_start(out=qt_sb, in_=qt_dram[:, :, t * P:(t + 1) * P].bitcast(F32R))
            gate_sb = p2.tile([P, D], F32, tag="gate")
            nc.sync.dma_start(out=gate_sb, in_=gate_dram[t])

            negmax4 = stats.tile([P, NB], F32, tag="negmax4")
            sums4 = stats.tile([P, NB], F32, tag="sums4")
            neg_max = stats.tile([P, 1], F32, tag="negmax")
            recip = stats.tile([P, 1], F32, tag="recip")
            sumx = stats.tile([P, 1], F32, tag="sumx")

            # scores + row stats
            score_ps = []
            for nb in range(NB):
                ps = ps_a.tile([P, 512], F32, tag="ps_a")
                for k in range(DK):
                    _mm(nc, ps, qt_sb[:, k, :],
                        kT_sb[:, k, nb * 512:(nb + 1) * 512],
                        start=(k == 0), stop=(k == DK - 1), r=True)
                nc.vector.tensor_reduce(
                    negmax4[:, nb:nb + 1], ps, axis=AX.X, op=ALU.max, negate=True)
                score_ps.append(ps)
            nc.vector.tensor_reduce(neg_max, negmax4, axis=AX.X, op=ALU.min)

            # exp(x - max) -> bf16 attn, accumulate row sums
            attn = p2.tile([P, S], BF16, tag="attn")
            for nb in range(NB):
                nc.scalar.activation(
                    attn[:, nb * 512:(nb + 1) * 512], score_ps[nb], ACTF.Exp,
                    bias=neg_max, accum_out=sums4[:, nb:nb + 1])
            nc.vector.tensor_reduce(sumx, sums4, axis=AX.X, op=ALU.add)
            nc.vector.reciprocal(recip, sumx)

            # transpose attention tile (bf16, PE)
            attnT = p2.tile([P, S], BF16, tag="attnT")
            for j in range(NB):
                pst = ps_a.tile([P, 512], BF16, tag="ps_a")
                for i in range(4):
                    kb = j * 4 + i
                    nc.tensor.transpose(
                        pst[:, i * P:(i + 1) * P],
                        attn[:, kb * P:(kb + 1) * P], ident_b)
                nc.vector.tensor_copy(attnT[:, j * 512:(j + 1) * 512], pst)

            # ctx = attnT x V, evict fused with 1/sum and gate
            ctxg = p2.tile([P, D], F32, tag="ctxg")
            for n in range(2):
                ps = ps_b.tile([P, 512], F32, tag="ps_b")
                for kb in range(ST):
                    _mm(nc, ps, attnT[:, kb * P:(kb + 1) * P],
                        v_sb[:, kb, n * 512:(n + 1) * 512],
                        start=(kb == 0), stop=(kb == ST - 1))
                nc.vector.scalar_tensor_tensor(
                    out=ctxg[:, n * 512:(n + 1) * 512], in0=ps, scalar=recip,
                    in1=gate_sb[:, n * 512:(n + 1) * 512],
                    op0=ALU.mult, op1=ALU.mult)

            # transpose ctxg (fp32, PE)
            ctxgT = p2.tile([P, DK, P], F32R, tag="ctxgT")
            for j in range(2):
                pst = ps_b.tile([P, 512], F32, tag="ps_b")
                for i in range(4):
                    c = j * 4 + i
                    nc.tensor.transpose(
                        pst[:, i * P:(i + 1) * P],
                        ctxg[:, c * P:(c + 1) * P], ident_f)
                nc.vector.tensor_copy(
                    ctxgT[:, j * 4:(j + 1) * 4, :],
                    pst.rearrange("p (a b) -> p a b", a=4))

            # out = ctxgT x Wo
            out_sb = p2.tile([P, D], F32, tag="out")
            for n in range(2):
                ps = ps_b.tile([P, 512], F32, tag="ps_b")
                for k in range(DK):
                    _mm(nc, ps, ctxgT[:, k, :],
                        wo_sb[:, k, n * 512:(n + 1) * 512],
                        start=(k == 0), stop=(k == DK - 1), r=True)
                dst = out_sb[:, n * 512:(n + 1) * 512]
                if bs is None:
                    nc.vector.tensor_copy(dst, ps)
                else:
                    nc.vector.scalar_tensor_tensor(
                        out=dst, in0=ps, scalar=1.0,
                        in1=bias_bc["bo"][:, n * 512:(n + 1) * 512],
                        op0=ALU.mult, op1=ALU.add)
            nc.sync.dma_start(out=out[t * P:(t + 1) * P, :], in_=out_sb)


_CACHE = {}


def _get_program(zero_bias: bool):
    if zero_bias not in _CACHE:
        _CACHE[zero_bias] = build_program(zero_bias)
    return _CACHE[zero_bias]


def kernel(queries, keys, values, Wq, bq, Wk, bk, Wv, bv, Wg, bg, Wo, bo):
    queries = np.ascontiguousarray(np.asarray(queries, dtype=np.float32))
    keys = np.ascontiguousarray(np.asarray(keys, dtype=np.float32))
    values = np.ascontiguousarray(np.asarray(values, dtype=np.float32))
    wdict = {
        "wq": np.ascontiguousarray(np.asarray(Wq, np.float32)),
        "wk": np.ascontiguousarray(np.asarray(Wk, np.float32)),
        "wv": np.ascontiguousarray(np.asarray(Wv, np.float32)),
        "wg": np.ascontiguousarray(np.asarray(Wg, np.float32)),
        "wo": np.ascontiguousarray(np.asarray(Wo, np.float32)),
    }
    bdict = {
        "bq": np.ascontiguousarray(np.asarray(bq, np.float32)),
        "bk": np.ascontiguousarray(np.asarray(bk, np.float32)),
        "bv": np.ascontiguousarray(np.asarray(bv, np.float32)),
        "bg": np.ascontiguousarray(np.asarray(bg, np.float32)),
        "bo": np.ascontiguousarray(np.asarray(bo, np.float32)),
    }
    zero_bias = all(not np.any(v) for v in bdict.values())
    nc = _get_program(zero_bias)

    in_maps = []
    for b in range(B):
        m = {"xq": queries[b], "xk": keys[b], "xv": values[b]}
        m.update(wdict)
        if not zero_bias:
            m.update(bdict)
        in_maps.append(m)
    res = run_bass_kernel_spmd(nc, in_maps, core_ids=list(range(B)))
    return np.stack([res.results[b]["out"] for b in range(B)], axis=0)
